# revision 24
# baseline (speedup 1.0000x reference)
"""Trainium2 Bass kernel: 4096x4096 valid cross-correlation with an 11x11
filter + scalar bias, sharded column-wise across 8 NeuronCores.

Strategy (v6: 64x64 PE array packing)
-------------------------------------
Host-side sharding as before: core c gets input columns [512c, 512c+522)
(core 7 shifted left), produces output columns [512c, 512c+512).

Per-core compute uses 4-way TensorE tiling: the 128x128 PE array is
packed as 4 independent 64x64 tiles (tile_position=(64r, 64c)). Each
tile contracts K=64 image rows against a banded stationary
    B_dj[k, 54dj + m] = w[k - m, dj]   (0 <= k - m < 11, m < 54)
producing 54 output rows; the 11 dj-shifted matmuls accumulate one PSUM
half-quadrant. 64x64 beats 32x32 here because every tiled matmul pays a
fixed ~30ns LDWEIGHTS sequencer cost plus a serialized (weight-load +
stream) on its tile, so fewer/taller matmuls win: 836 MMs vs 2068.

A wave = 108 output rows = 2 blocks of 54. Block r of wave w lives in
SBUF partition group r (input rows 108w+54r .. +64, 10-row halo overlap
between groups) and computes on tile (r, c=(r+w)%2); the rotation makes
2 consecutive waves occupy all 4 tiles. Each wave accumulates in one
PSUM bank, then ACT/DVE (alternating) copy PSUM->SBUF as bf16. Output
goes to DRAM in the permuted [partition, wave, col] layout and is
unpermuted on the host (index gather + scalar bias add).

The whole per-core input (38 waves x 522 cols bf16 = 40KB/partition) and
output (39KB/partition) stay resident in SBUF, so I/O is a handful of
megabyte-scale DMAs: 8 overlapped-row input loads (3-dim APs, one per
partition group per chunk), 6 output stores, all issued early enough to
overlap compute.
"""

import os
import sys

import numpy as np

for _p in ("/opt/trn_rl_repo", "/root/.axon_site/_ro/trn_rl_repo"):
    if os.path.isdir(_p) and _p not in sys.path:
        sys.path.insert(0, _p)

# The device run goes through jax's axon PJRT backend; make sure it is
# visible if jax has not been initialized yet.
_jp = os.environ.get("JAX_PLATFORMS", "")
if "axon" not in _jp.split(","):
    os.environ["JAX_PLATFORMS"] = ("axon," + _jp).strip(",")

import ml_dtypes

import concourse.bacc as bacc
import concourse.bass as bass
import concourse.mybir as mybir
import concourse.tile as tile
from concourse.bass import AP
from concourse.bass_utils import run_bass_kernel_spmd

H = W = 4096
KH = KW = 11
OH = OW = H - KH + 1  # 4086
NCORES = 8
COLS_OUT = 512            # output columns per core
COLS_IN = COLS_OUT + KW - 1  # 522
BL = 54                   # output rows per 64x64 tile (K=64 minus 10 halo)
WV = 2 * BL               # 108 output rows per wave
NWAVE = 38                # 37 full waves + overlapping tail wave
WBASE = [108 * w for w in range(37)] + [OH - WV]  # tail at 3978
XF = COLS_IN              # free elements per wave in xAll
OF = COLS_OUT             # free elements per wave in otAll

_cache: dict = {}
LAST_RESULT = None  # BassKernelResults of the most recent device run

# host unpermute maps: output row -> (source partition, source wave)
_SRC_P = np.empty(OH, np.int64)
_SRC_W = np.empty(OH, np.int64)
for _w in range(NWAVE):
    _base = WBASE[_w]
    for _r in range(2):
        _c = (_r + _w) % 2
        _rows = np.arange(_base + BL * _r, _base + BL * _r + BL)
        _SRC_P[_rows] = 64 * _c + np.arange(BL)
        _SRC_W[_rows] = _w


def _build():
    f32 = mybir.dt.float32
    bf16 = mybir.dt.bfloat16
    nc = bacc.Bacc("TRN2", target_bir_lowering=False, debug=False,
                   num_devices=NCORES)
    xs_d = nc.dram_tensor("xs", [H, COLS_IN], bf16, kind="ExternalInput")
    bd_d = nc.dram_tensor("bands", [128, KW * BL], bf16,
                          kind="ExternalInput")
    op_d = nc.dram_tensor("operm", [128, NWAVE * OF], bf16,
                          kind="ExternalOutput")

    with tile.TileContext(nc) as tc:
        with (
            tc.tile_pool(name="bp", bufs=1) as bp,
            tc.tile_pool(name="xp", bufs=1) as xp,
            tc.tile_pool(name="op", bufs=1) as op,
            tc.tile_pool(name="pp", bufs=8, space=bass.MemorySpace.PSUM) as pp,
        ):
            xAll = xp.tile([128, NWAVE * XF], bf16, name="xAll")
            otAll = op.tile([128, NWAVE * OF], bf16, name="otAll")
            bt = bp.tile([128, KW * BL], bf16, name="bt")

            # bands first (first matmul needs them), then input chunks in
            # compute order. Overlapped-row source APs: partition group g
            # of wave w holds image rows 108w+54g .. +64. DMA AP balancing
            # caps at 3 dims, so one DMA per (chunk, partition group).
            nc.sync.dma_start(bt[:], bd_d.ap()[:, :])
            in_chunks = [(0, 2), (2, 8), (8, 24), (24, 37)]
            for (w0, w1) in in_chunks:
                n = w1 - w0
                for g in range(2):
                    src = AP(xs_d, (108 * w0 + BL * g) * COLS_IN,
                             [(COLS_IN, 64), (108 * COLS_IN, n),
                              (1, COLS_IN)])
                    if w0 == 2:
                        # waves 2-7 are needed ~5us after compute starts,
                        # before the serialized HWDGE streams can deliver
                        # them; SWDGE sprays this chunk across all 16 SDMA
                        # engines in parallel with the HWDGE triggers.
                        eng = nc.gpsimd
                    else:
                        eng = nc.sync if g == 0 else nc.scalar
                    eng.dma_start(xAll[64 * g:64 * g + 64, XF * w0:XF * w1],
                                  src)
            src = AP(xs_d, WBASE[37] * COLS_IN,
                     [(BL * COLS_IN, 2), (COLS_IN, 64), (1, COLS_IN)])
            nc.sync.dma_start(xAll[:, XF * 37:XF * 38], src)

            # store chunks: emitted as soon as their wave range is done so
            # stores overlap compute; both sides contiguous per partition.
            st_bounds = [0, 8, 16, 24, 32, 35, 36, 37, NWAVE]
            st_engines = [nc.gpsimd, nc.sync, nc.scalar]
            st_next = 1

            # 4-wave groups, dj-inner: per dj round the 8 matmuls cover
            # all 4 tiles twice; consecutive instructions hit distinct
            # tiles so dispatch stalls only on the wrap, which matches the
            # tile-serial (weight-load + stream) period anyway.
            groups = [list(range(4 * g, min(4 * g + 4, NWAVE)))
                      for g in range((NWAVE + 3) // 4)]
            for waves in groups:
                pts = {w: pp.tile([128, OF], f32, tag="ps", name=f"ps{w}")
                       for w in waves}
                for dj in range(KW):
                    for w in waves:
                        for r in range(2):
                            c = (r + w) % 2
                            nc.tensor.matmul(
                                pts[w][64 * c:64 * c + BL, :],
                                bt[64 * r:64 * r + 64,
                                   BL * dj:BL * dj + BL],
                                xAll[64 * r:64 * r + 64,
                                     XF * w + dj:XF * w + dj + COLS_OUT],
                                start=(dj == 0),
                                stop=(dj == KW - 1),
                                tile_position=(64 * r, 64 * c),
                            )
                for w in waves:
                    if w % 2 == 0:
                        nc.scalar.copy(otAll[:, OF * w:OF * w + OF],
                                       pts[w][:, :])
                    else:
                        nc.vector.tensor_copy(otAll[:, OF * w:OF * w + OF],
                                              pts[w][:, :])
                done = waves[-1] + 1
                while st_next < len(st_bounds) and st_bounds[st_next] <= done:
                    w0, w1 = st_bounds[st_next - 1], st_bounds[st_next]
                    eng = st_engines[st_next % 3]
                    eng.dma_start(op_d.ap()[:, OF * w0:OF * w1],
                                  otAll[:, OF * w0:OF * w1])
                    st_next += 1
    nc.compile()
    return nc


def _bands_from_weight(weight: np.ndarray) -> np.ndarray:
    b = np.zeros((128, KW * BL), np.float32)
    for r in range(2):
        for dj in range(KW):
            for m in range(BL):
                b[64 * r + m:64 * r + m + KH, BL * dj + m] = weight[:, dj]
    return b.astype(ml_dtypes.bfloat16)


def kernel(x: np.ndarray, weight: np.ndarray, bias: np.ndarray,
           _trace: bool = False, **_trace_kwargs) -> np.ndarray:
    global LAST_RESULT
    x = np.asarray(x, dtype=np.float32)
    weight = np.asarray(weight, dtype=np.float32)
    bias_v = float(np.asarray(bias, dtype=np.float32).reshape(-1)[0])

    if "nc" not in _cache:
        _cache["nc"] = _build()
    nc = _cache["nc"]

    bands = _bands_from_weight(weight)
    xb = x.astype(ml_dtypes.bfloat16)
    starts = [min(c * COLS_OUT, W - COLS_IN) for c in range(NCORES)]
    in_maps = [
        {"xs": np.ascontiguousarray(xb[:, s:s + COLS_IN]), "bands": bands}
        for s in starts
    ]
    res = run_bass_kernel_spmd(nc, in_maps, core_ids=list(range(NCORES)),
                               trace=_trace, **_trace_kwargs)
    LAST_RESULT = res

    out = np.empty((OH, OW), dtype=np.float32)
    for cc, s in enumerate(starts):
        perm = np.asarray(res.results[cc]["operm"]).reshape(128, NWAVE, OF)
        core_out = perm[_SRC_P, _SRC_W, :].astype(np.float32)
        g0 = cc * COLS_OUT          # first global output col from core cc
        keep0 = g0 - s              # 0 for cores 0-6, 10 for core 7
        take = min(COLS_OUT - keep0, OW - g0)
        out[:, g0:g0 + take] = core_out[:, keep0:keep0 + take]
    if bias_v != 0.0:
        out += bias_v
    return out


# revision 25
# speedup vs baseline: 1.2239x; 1.2239x over previous
"""Trainium2 Bass kernel: 4096x4096 valid cross-correlation with an 11x11
filter + scalar bias, sharded column-wise across 8 NeuronCores.

Strategy (v6: 64x64 PE array packing)
-------------------------------------
Host-side sharding as before: core c gets input columns [512c, 512c+522)
(core 7 shifted left), produces output columns [512c, 512c+512).

Per-core compute uses 4-way TensorE tiling: the 128x128 PE array is
packed as 4 independent 64x64 tiles (tile_position=(64r, 64c)). Each
tile contracts K=64 image rows against a banded stationary
    B_dj[k, 54dj + m] = w[k - m, dj]   (0 <= k - m < 11, m < 54)
producing 54 output rows; the 11 dj-shifted matmuls accumulate one PSUM
half-quadrant. 64x64 beats 32x32 here because every tiled matmul pays a
fixed ~30ns LDWEIGHTS sequencer cost plus a serialized (weight-load +
stream) on its tile, so fewer/taller matmuls win: 836 MMs vs 2068.

A wave = 108 output rows = 2 blocks of 54. Block r of wave w lives in
SBUF partition group r (input rows 108w+54r .. +64, 10-row halo overlap
between groups) and computes on tile (r, c=(r+w)%2); the rotation makes
2 consecutive waves occupy all 4 tiles. Each wave accumulates in one
PSUM bank, then ACT/DVE (alternating) copy PSUM->SBUF as bf16. Output
goes to DRAM in the permuted [partition, wave, col] layout and is
unpermuted on the host (index gather + scalar bias add).

The whole per-core input (38 waves x 522 cols bf16 = 40KB/partition) and
output (39KB/partition) stay resident in SBUF, so I/O is a handful of
megabyte-scale DMAs: 8 overlapped-row input loads (3-dim APs, one per
partition group per chunk), 6 output stores, all issued early enough to
overlap compute.
"""

import os
import sys

import numpy as np

for _p in ("/opt/trn_rl_repo", "/root/.axon_site/_ro/trn_rl_repo"):
    if os.path.isdir(_p) and _p not in sys.path:
        sys.path.insert(0, _p)

# The device run goes through jax's axon PJRT backend; make sure it is
# visible if jax has not been initialized yet.
_jp = os.environ.get("JAX_PLATFORMS", "")
if "axon" not in _jp.split(","):
    os.environ["JAX_PLATFORMS"] = ("axon," + _jp).strip(",")

import ml_dtypes

import concourse.bacc as bacc
import concourse.bass as bass
import concourse.mybir as mybir
import concourse.tile as tile
from concourse.bass import AP
from concourse.bass_utils import run_bass_kernel_spmd

H = W = 4096
KH = KW = 11
OH = OW = H - KH + 1  # 4086
NCORES = 8
COLS_OUT = 512            # output columns per core
COLS_IN = COLS_OUT + KW - 1  # 522
BL = 54                   # output rows per 64x64 tile (K=64 minus 10 halo)
WV = 2 * BL               # 108 output rows per wave
NWAVE = 38                # 37 full waves + overlapping tail wave
WBASE = [108 * w for w in range(37)] + [OH - WV]  # tail at 3978
XF = COLS_IN              # free elements per wave in xAll
OF = COLS_OUT             # free elements per wave in otAll

_cache: dict = {}
LAST_RESULT = None  # BassKernelResults of the most recent device run

# host unpermute maps: output row -> (source partition, source wave)
_SRC_P = np.empty(OH, np.int64)
_SRC_W = np.empty(OH, np.int64)
for _w in range(NWAVE):
    _base = WBASE[_w]
    for _r in range(2):
        _c = (_r + _w) % 2
        _rows = np.arange(_base + BL * _r, _base + BL * _r + BL)
        _SRC_P[_rows] = 64 * _c + np.arange(BL)
        _SRC_W[_rows] = _w


def _build():
    f32 = mybir.dt.float32
    bf16 = mybir.dt.bfloat16
    nc = bacc.Bacc("TRN2", target_bir_lowering=False, debug=False,
                   num_devices=NCORES)
    xs_d = nc.dram_tensor("xs", [H, COLS_IN], bf16, kind="ExternalInput")
    bd_d = nc.dram_tensor("bands", [128, KW * BL], bf16,
                          kind="ExternalInput")
    op_d = nc.dram_tensor("operm", [128, NWAVE * OF], bf16,
                          kind="ExternalOutput")

    with tile.TileContext(nc) as tc:
        with (
            tc.tile_pool(name="bp", bufs=1) as bp,
            tc.tile_pool(name="xp", bufs=1) as xp,
            tc.tile_pool(name="op", bufs=1) as op,
            tc.tile_pool(name="pp", bufs=8, space=bass.MemorySpace.PSUM) as pp,
        ):
            xAll = xp.tile([128, NWAVE * XF], bf16, name="xAll")
            otAll = op.tile([128, NWAVE * OF], bf16, name="otAll")
            bt = bp.tile([128, KW * BL], bf16, name="bt")

            # bands first (first matmul needs them), then input chunks in
            # compute order. Overlapped-row source APs: partition group g
            # of wave w holds image rows 108w+54g .. +64. DMA AP balancing
            # caps at 3 dims, so one DMA per (chunk, partition group).
            nc.sync.dma_start(bt[:], bd_d.ap()[:, :])
            in_chunks = [(0, 2), (2, 8), (8, 24), (24, 37)]
            for (w0, w1) in in_chunks:
                n = w1 - w0
                for g in range(2):
                    src = AP(xs_d, (108 * w0 + BL * g) * COLS_IN,
                             [(COLS_IN, 64), (108 * COLS_IN, n),
                              (1, COLS_IN)])
                    eng = nc.sync if g == 0 else nc.scalar
                    eng.dma_start(xAll[64 * g:64 * g + 64, XF * w0:XF * w1],
                                  src)
            src = AP(xs_d, WBASE[37] * COLS_IN,
                     [(BL * COLS_IN, 2), (COLS_IN, 64), (1, COLS_IN)])
            nc.sync.dma_start(xAll[:, XF * 37:XF * 38], src)

            # store chunks: emitted as soon as their wave range is done so
            # stores overlap compute; both sides contiguous per partition.
            st_bounds = [0, 8, 16, 24, 32, 35, 36, 37, NWAVE]
            st_engines = [nc.gpsimd, nc.sync, nc.scalar]
            st_next = 1

            # 4-wave groups, dj-inner: per dj round the 8 matmuls cover
            # all 4 tiles twice; consecutive instructions hit distinct
            # tiles so dispatch stalls only on the wrap, which matches the
            # tile-serial (weight-load + stream) period anyway.
            groups = [list(range(4 * g, min(4 * g + 4, NWAVE)))
                      for g in range((NWAVE + 3) // 4)]
            for waves in groups:
                pts = {w: pp.tile([128, OF], f32, tag="ps", name=f"ps{w}")
                       for w in waves}
                for dj in range(KW):
                    for w in waves:
                        for r in range(2):
                            c = (r + w) % 2
                            nc.tensor.matmul(
                                pts[w][64 * c:64 * c + BL, :],
                                bt[64 * r:64 * r + 64,
                                   BL * dj:BL * dj + BL],
                                xAll[64 * r:64 * r + 64,
                                     XF * w + dj:XF * w + dj + COLS_OUT],
                                start=(dj == 0),
                                stop=(dj == KW - 1),
                                tile_position=(64 * r, 64 * c),
                            )
                for w in waves:
                    if w % 2 == 0:
                        nc.scalar.copy(otAll[:, OF * w:OF * w + OF],
                                       pts[w][:, :])
                    else:
                        nc.vector.tensor_copy(otAll[:, OF * w:OF * w + OF],
                                              pts[w][:, :])
                done = waves[-1] + 1
                while st_next < len(st_bounds) and st_bounds[st_next] <= done:
                    w0, w1 = st_bounds[st_next - 1], st_bounds[st_next]
                    eng = st_engines[st_next % 3]
                    eng.dma_start(op_d.ap()[:, OF * w0:OF * w1],
                                  otAll[:, OF * w0:OF * w1])
                    st_next += 1
    nc.compile()
    return nc


def _bands_from_weight(weight: np.ndarray) -> np.ndarray:
    b = np.zeros((128, KW * BL), np.float32)
    for r in range(2):
        for dj in range(KW):
            for m in range(BL):
                b[64 * r + m:64 * r + m + KH, BL * dj + m] = weight[:, dj]
    return b.astype(ml_dtypes.bfloat16)


def kernel(x: np.ndarray, weight: np.ndarray, bias: np.ndarray,
           _trace: bool = False, **_trace_kwargs) -> np.ndarray:
    global LAST_RESULT
    x = np.asarray(x, dtype=np.float32)
    weight = np.asarray(weight, dtype=np.float32)
    bias_v = float(np.asarray(bias, dtype=np.float32).reshape(-1)[0])

    if "nc" not in _cache:
        _cache["nc"] = _build()
    nc = _cache["nc"]

    bands = _bands_from_weight(weight)
    xb = x.astype(ml_dtypes.bfloat16)
    starts = [min(c * COLS_OUT, W - COLS_IN) for c in range(NCORES)]
    in_maps = [
        {"xs": np.ascontiguousarray(xb[:, s:s + COLS_IN]), "bands": bands}
        for s in starts
    ]
    res = run_bass_kernel_spmd(nc, in_maps, core_ids=list(range(NCORES)),
                               trace=_trace, **_trace_kwargs)
    LAST_RESULT = res

    out = np.empty((OH, OW), dtype=np.float32)
    for cc, s in enumerate(starts):
        perm = np.asarray(res.results[cc]["operm"]).reshape(128, NWAVE, OF)
        core_out = perm[_SRC_P, _SRC_W, :].astype(np.float32)
        g0 = cc * COLS_OUT          # first global output col from core cc
        keep0 = g0 - s              # 0 for cores 0-6, 10 for core 7
        take = min(COLS_OUT - keep0, OW - g0)
        out[:, g0:g0 + take] = core_out[:, keep0:keep0 + take]
    if bias_v != 0.0:
        out += bias_v
    return out
